# revision 10
# baseline (speedup 1.0000x reference)
"""Trainium2 Bass kernel for nn_HardNegativeWeightedFConLoss.

8-core SPMD: each core owns 512 anchor rows (of 4096). Replicated embM/embP
(+ host-transposed copies) and MLP params; per-row topk handled via a
moment-initialized Newton threshold search in the exp-sim domain with fused
count/sum DVE passes. Host only reshapes/transposes/shards inputs and
averages the 8 partial loss sums.
"""
import sys
sys.path.insert(0, "/opt/trn_rl_repo")
import numpy as np

import concourse.bass as bass
import concourse.bacc as bacc
import concourse.mybir as mybir
from concourse.tile import TileContext

F32 = mybir.dt.float32
F32R = mybir.dt.float32r
BF16 = mybir.dt.bfloat16
U8 = mybir.dt.uint8
I32 = mybir.dt.int32
AF = mybir.ActivationFunctionType
ALU = mybir.AluOpType
AX = mybir.AxisListType

TAU = 0.1
WEIGHT_TEMP = 0.7
HARD_RATIO = 0.3
# probit constants at p=0.3 (z s.t. Phi(z)=0.7): threshold init t0 = mu + sig*z(p)
Z0 = 0.5244005127080409
PHI0 = 0.3476926          # N(0,1) pdf at Z0
DZDP = -1.0 / PHI0
N_NEWTON = 3              # count/update iterations before the final select-sum

P = 128


def build(R, N, D, n_newton=N_NEWTON):
    """Build the per-core Bass program. R = anchor rows per core."""
    NB = R // P          # row blocks
    JC = N // 512        # 512-wide column chunks of the sim matrix
    KC = D // P          # 128-deep contraction chunks over d
    HB = D // P          # hidden blocks (MLP hidden dim == D)
    FK = 2 * D // P      # feats contraction chunks

    nc = bacc.Bacc()
    dp = nc.declare_dram_parameter
    embF = dp("embF", [R, D], F32, isOutput=False)
    embMT = dp("embMT", [D, N], F32, isOutput=False)
    embPT = dp("embPT", [D, N], F32, isOutput=False)
    embM_hi = dp("embM_hi", [N, D], BF16, isOutput=False)
    embM_lo = dp("embM_lo", [N, D], BF16, isOutput=False)
    embP_hi = dp("embP_hi", [N, D], BF16, isOutput=False)
    embP_lo = dp("embP_lo", [N, D], BF16, isOutput=False)
    adjTM = dp("adjTM", [N, R], BF16, isOutput=False)
    adjTP = dp("adjTP", [N, R], BF16, isOutput=False)
    negadjM = dp("negadjM", [R, N], BF16, isOutput=False)
    negadjP = dp("negadjP", [R, N], BF16, isOutput=False)
    W1 = dp("W1", [2 * D, D], F32, isOutput=False)
    b1 = dp("b1", [D, 1], F32, isOutput=False)
    W2 = dp("W2", [D, 2], F32, isOutput=False)
    b2 = dp("b2", [2, 1], F32, isOutput=False)
    ident = dp("ident", [P, P], F32, isOutput=False)
    loss_out = dp("loss_rows", [R, 1], F32, isOutput=True)
    wts_out = dp("wtsT", [2, R], F32, isOutput=True)


    with TileContext(nc) as tc:
        with (
            tc.tile_pool(name="persist", bufs=1) as p_per,     # ident/fnt/stats
            tc.tile_pool(name="stream", bufs=1) as p_st,       # rotating loads
            tc.tile_pool(name="small", bufs=2) as p_sm,        # [128,<=8] scalars
            tc.tile_pool(name="row", bufs=1) as p_row,         # [<=2, <=512] rows
            tc.tile_pool(name="scr", bufs=2) as p_scr,         # [128,512] scratch
            tc.tile_pool(name="psum", bufs=1, space="PSUM") as ps,
        ):
            BIG = dict(tag="big", bufs=5)        # [128,512] psum slots
            SMALL = dict(tag="small", bufs=2)    # [<=128,<=512]-ish psum slots

            # ---------- constants ----------
            id_t = p_per.tile([P, P], F32, tag="ident")
            nc.sync.dma_start(id_t[:], ident[:])
            ones_k1 = p_per.tile([P, 1], F32, tag="ones_k1")
            nc.vector.memset(ones_k1[:], 1.0)
            ones_1m = p_per.tile([1, P], F32, tag="ones_1m")
            nc.vector.memset(ones_1m[:], 1.0)
            ones_kb = p_per.tile([P, 1], BF16, tag="ones_kb")
            nc.vector.memset(ones_kb[:], 1.0)

            # ---------- Phase 0: FnT (normalized, 1/TAU folded, transposed) ----------
            fnt = [p_per.tile([P, R], BF16, tag=f"fnt{kc}", name=f"fnt{kc}") for kc in range(KC)]
            for b in range(NB):
                eb = p_st.tile([P, D], F32, tag="ldF", bufs=2)
                nc.sync.dma_start(eb[:], embF[b * P:(b + 1) * P, :])
                nsq = p_sm.tile([P, 1], F32, tag="fn_nsq")
                sc = p_scr.tile([P, D], F32, tag="scr")
                nc.vector.scalar_tensor_tensor(
                    out=sc[:], in0=eb[:], scalar=1.0, in1=eb[:],
                    op0=ALU.mult, op1=ALU.mult, accum_out=nsq[:])
                nrm = p_sm.tile([P, 1], F32, tag="fn_nrm")
                nc.scalar.activation(nrm[:], nsq[:], AF.Sqrt)
                nc.vector.tensor_scalar_max(nrm[:], nrm[:], 1e-12)
                rn = p_sm.tile([P, 1], F32, tag="fn_rn")
                nc.vector.reciprocal(rn[:], nrm[:])
                nc.vector.tensor_scalar(out=rn[:], in0=rn[:], scalar1=1.0 / TAU,
                                        scalar2=None, op0=ALU.mult)
                fb = p_st.tile([P, D], F32, tag="fnb", bufs=2)
                nc.vector.tensor_scalar(out=fb[:], in0=eb[:], scalar1=rn[:],
                                        scalar2=None, op0=ALU.mult)
                for kc in range(KC):
                    pt = ps.tile([P, P], F32, **SMALL)
                    nc.tensor.transpose(pt[:], fb[:, kc * P:(kc + 1) * P], id_t[:])
                    nc.scalar.copy(fnt[kc][:, b * P:(b + 1) * P], pt[:])

            wblk = []
            with tc.tile_pool(name="mlp", bufs=1) as p_mlp:
                # ---------- Phase A: aggregation reprT = (adj @ emb).T / deg ----------
                feats = [p_mlp.tile([P, R], F32, tag=f"feats{i}", name=f"feats{i}") for i in range(FK)]
                for mi, (ehi, elo, adjTX) in enumerate(
                        ((embM_hi, embM_lo, adjTM), (embP_hi, embP_lo, adjTP))):
                    repr_ps = [ps.tile([P, R], F32, name=f"repr{dc}", **BIG) for dc in range(KC)]
                    deg_ps = ps.tile([1, R], F32, **SMALL)
                    njb = N // P
                    for jb in range(njb):
                        eth = p_st.tile([P, D], BF16, tag="ldEh", bufs=3)
                        nc.sync.dma_start(eth[:], ehi[jb * P:(jb + 1) * P, :])
                        etl = p_st.tile([P, D], BF16, tag="ldEl", bufs=3)
                        nc.sync.dma_start(etl[:], elo[jb * P:(jb + 1) * P, :])
                        at = p_st.tile([P, R], BF16, tag="ldA", bufs=3)
                        nc.sync.dma_start(at[:], adjTX[jb * P:(jb + 1) * P, :])
                        for dc in range(KC):
                            nc.tensor.matmul(repr_ps[dc][:], eth[:, dc * P:(dc + 1) * P],
                                             at[:], start=(jb == 0), stop=False)
                            nc.tensor.matmul(repr_ps[dc][:], etl[:, dc * P:(dc + 1) * P],
                                             at[:], start=False, stop=(jb == njb - 1))
                        nc.tensor.matmul(deg_ps[:], ones_kb[:], at[:],
                                         start=(jb == 0), stop=(jb == njb - 1))
                    # 1/max(deg,1) broadcast to all partitions
                    dg = p_row.tile([1, R], F32, tag="degsb")
                    nc.vector.tensor_scalar_max(dg[:], deg_ps[:], 1.0)
                    rdg = p_row.tile([1, R], F32, tag="rdeg")
                    nc.vector.reciprocal(rdg[:], dg[:])
                    bc_ps = ps.tile([P, R], F32, **BIG)
                    nc.tensor.matmul(bc_ps[:], (ones_1m[:]), (rdg[:]),
                                     start=True, stop=True)
                    bcd = p_scr.tile([P, R], F32, tag="bcd_sb")
                    nc.scalar.copy(bcd[:], bc_ps[:])
                    for dc in range(KC):
                        nc.vector.tensor_tensor(feats[mi * KC + dc][:], repr_ps[dc][:],
                                                bcd[:], ALU.mult)

                # ---------- Phase B: MLP + double softmax ----------
                w1t = [p_mlp.tile([P, D], F32, tag=f"w1_{kc}", name=f"w1_{kc}") for kc in range(FK)]
                for kc in range(FK):
                    nc.sync.dma_start(w1t[kc][:], W1[kc * P:(kc + 1) * P, :])
                b1t = p_per.tile([P, HB], F32, tag="b1")
                nc.sync.dma_start(b1t[:].rearrange("p (h o) -> p h o", o=1),
                  b1[:].rearrange("(h p) o -> p h o", p=P))
                w2t = p_per.tile([P, KC * 2], F32, tag="w2")
                nc.sync.dma_start(w2t[:].rearrange("p (h o) -> p h o", o=2),
                  W2[:].rearrange("(h p) o -> p h o", p=P))
                b2t = []
                for o in range(2):
                    b2o = p_per.tile([1, 1], F32, tag=f"b2_{o}", name=f"b2_{o}")
                    nc.sync.dma_start(b2o[:], b2[o:o + 1, :])
                    b2t.append(b2o)

                ht = [p_mlp.tile([P, R], F32, tag=f"ht{h}", name=f"ht{h}") for h in range(HB)]
                for h in range(HB):
                    hps = ps.tile([P, R], F32, **BIG)
                    for kc in range(FK):
                        nc.tensor.matmul(hps[:], (w1t[kc][:, h * P:(h + 1) * P]),
                                         (feats[kc][:]), start=(kc == 0),
                                         stop=(kc == FK - 1))
                    nc.scalar.activation(ht[h][:], hps[:], AF.Relu, bias=b1t[:, h:h + 1])
                lgs = []
                for o in range(2):
                    lg_ps = ps.tile([1, R], F32, name="lg_ps", **SMALL)
                    for h in range(HB):
                        nc.tensor.matmul(lg_ps[:], (w2t[:, 2 * h + o:2 * h + o + 1]),
                                         (ht[h][:]), start=(h == 0), stop=(h == HB - 1))
                    lg = p_row.tile([1, R], F32, tag=f"lgsb{o}")
                    nc.scalar.activation(lg[:], lg_ps[:], AF.Identity, bias=b2t[o][:])
                    lgs.append(lg)

                def softmax2(src0, src1, inv_temp, tg):
                    mx = p_row.tile([1, R], F32, tag="sm_mx", bufs=2)
                    nc.vector.tensor_tensor(mx[:], src0[:], src1[:], ALU.max)
                    es = []
                    for r_, sr in enumerate((src0, src1)):
                        d_ = p_row.tile([1, R], F32, tag="sm_d", bufs=2)
                        nc.vector.tensor_tensor(d_[:], sr[:], mx[:], ALU.subtract)
                        e_ = p_row.tile([1, R], F32, tag=f"{tg}_e{r_}")
                        nc.scalar.activation(e_[:], d_[:], AF.Exp, scale=inv_temp)
                        es.append(e_)
                    s_ = p_row.tile([1, R], F32, tag="sm_s", bufs=2)
                    nc.vector.tensor_tensor(s_[:], es[0][:], es[1][:], ALU.add)
                    rs = p_row.tile([1, R], F32, tag="sm_rs", bufs=2)
                    nc.vector.reciprocal(rs[:], s_[:])
                    outs = []
                    for r_ in range(2):
                        o_ = p_row.tile([1, R], F32, tag=f"{tg}_o{r_}")
                        nc.vector.tensor_tensor(o_[:], es[r_][:], rs[:], ALU.mult)
                        outs.append(o_)
                    return outs

                raw0, raw1 = softmax2(lgs[0], lgs[1], 1.0, "sma")
                wts0, wts1 = softmax2(raw0, raw1, 1.0 / WEIGHT_TEMP, "smb")
                nc.sync.dma_start(wts_out[0:1, :], wts0[:])
                nc.sync.dma_start(wts_out[1:2, :], wts1[:])
                # per-row layout [128,2] per block via PE transpose (w1 parked at
                # partition 32 -- engine APs must start at partition 0/32/64/96)
                wpad = p_scr.tile([P, R], F32, tag="wpad", bufs=1)
                nc.vector.memset(wpad[:], 0.0)
                nc.scalar.copy(wpad[0:1, :], wts0[:])
                nc.scalar.copy(wpad[32:33, :], wts1[:])
                for b in range(NB):
                    pt = ps.tile([P, P], F32, **SMALL)
                    nc.tensor.transpose(pt[:], wpad[:, b * P:(b + 1) * P], id_t[:])
                    wb = p_per.tile([P, 2], F32, tag=f"wblk{b}")
                    nc.scalar.copy(wb[:, 0:1], pt[:, 0:1])
                    nc.scalar.copy(wb[:, 1:2], pt[:, 32:33])
                    wblk.append(wb)

            # ---------- Phase C/D: sims + topk selection ----------
            stats = {}  # (mi, b) -> dict of [128,1] tiles
            with (
                tc.tile_pool(name="mnt", bufs=1) as p_mnt,
                tc.tile_pool(name="vneg", bufs=2) as p_vneg,
                tc.tile_pool(name="nadj", bufs=2) as p_nadj,
                tc.tile_pool(name="cnt8", bufs=2) as p_cnt8,
                tc.tile_pool(name="vt", bufs=3) as p_v,
            ):
                for mi, (embXT, negadjX) in enumerate(((embMT, negadjM), (embPT, negadjP))):
                    # — build normalized bf16 (emb/|emb|).T resident in SBUF —
                    # stream embXT in [128,512] f32 chunks per column block: normsq
                    # via ones-matmul, then scale+round the same chunks to bf16
                    mnt = [p_mnt.tile([P, N], BF16, tag=f"mnt{dc}", name=f"mnt{dc}")
                           for dc in range(KC)]
                    for jc in range(JC):
                        chs = []
                        nsq_ps = ps.tile([1, 512], F32, **SMALL)
                        for dc in range(KC):
                            ch = p_st.tile([P, 512], F32, tag="mfa", bufs=KC + 1,
                                           name="ch")
                            nc.sync.dma_start(
                                ch[:], embXT[dc * P:(dc + 1) * P, jc * 512:(jc + 1) * 512])
                            chs.append(ch)
                            sq = p_scr.tile([P, 512], BF16, tag="scrb")
                            nc.vector.tensor_tensor(sq[:], ch[:], ch[:], ALU.mult)
                            nc.tensor.matmul(nsq_ps[:], ones_kb[:], sq[:],
                                             start=(dc == 0), stop=(dc == KC - 1))
                        rn_ = p_row.tile([1, 512], F32, tag="rn_", bufs=2)
                        nc.scalar.activation(rn_[:], nsq_ps[:], AF.Sqrt)
                        nc.vector.tensor_scalar_max(rn_[:], rn_[:], 1e-12)
                        nc.vector.reciprocal(rn_[:], rn_[:])
                        bc_ps = ps.tile([P, 512], F32, **BIG)
                        nc.tensor.matmul(bc_ps[:], ones_1m[:], rn_[:],
                                         start=True, stop=True)
                        bcn = p_scr.tile([P, 512], F32, tag="bcn_sb")
                        nc.scalar.copy(bcn[:], bc_ps[:])
                        for dc in range(KC):
                            nc.vector.tensor_tensor(mnt[dc][:, jc * 512:(jc + 1) * 512],
                                                    chs[dc][:], bcn[:], ALU.mult)

                    # — per row-block: sims, masked sums, Newton top-k —
                    for b in range(NB):
                        na = p_nadj.tile([P, N], BF16, tag="nadj")
                        nc.sync.dma_start(na[:], negadjX[b * P:(b + 1) * P, :])
                        vneg = p_vneg.tile([P, N], BF16, tag="vneg")
                        tot_c = p_sm.tile([P, JC], F32, tag="tot_c")
                        neg_c = p_sm.tile([P, JC], F32, tag="neg_c")
                        m2_c = p_sm.tile([P, JC], F32, tag="m2_c")
                        for jc in range(JC):
                            sps = ps.tile([P, 512], F32, **BIG)
                            for kc in range(KC):
                                nc.tensor.matmul(sps[:], (fnt[kc][:, b * P:(b + 1) * P]),
                                                 (mnt[kc][:, jc * 512:(jc + 1) * 512]),
                                                 start=(kc == 0), stop=(kc == KC - 1))
                            v = p_v.tile([P, 512], F32, tag="v")
                            nc.scalar.activation(v[:], sps[:], AF.Exp,
                                                 accum_out=tot_c[:, jc:jc + 1])
                            vn_sl = vneg[:, jc * 512:(jc + 1) * 512]
                            nc.vector.scalar_tensor_tensor(
                                out=vn_sl, in0=v[:], scalar=1.0,
                                in1=na[:, jc * 512:(jc + 1) * 512],
                                op0=ALU.mult, op1=ALU.mult,
                                accum_out=neg_c[:, jc:jc + 1])
                            sc2 = p_scr.tile([P, 512], F32, tag="scr")
                            nc.vector.scalar_tensor_tensor(
                                out=sc2[:], in0=v[:], scalar=1.0, in1=vn_sl,
                                op0=ALU.mult, op1=ALU.mult, accum_out=m2_c[:, jc:jc + 1])
                        st = {}
                        for nm, cols in (("tot", tot_c), ("neg", neg_c), ("m2", m2_c)):
                            t_ = p_per.tile([P, 1], F32, tag=f"st_{nm}_{mi}_{b}")
                            nc.vector.tensor_reduce(t_[:], cols[:], AX.X, ALU.add)
                            st[nm] = t_
                        nn = p_per.tile([P, 1], F32, tag=f"st_nn_{mi}_{b}")
                        nc.vector.tensor_reduce(nn[:], na[:], AX.X, ALU.add)
                        st["nneg"] = nn
                        # k = max(trunc(nneg * 0.3), 1)
                        kf = p_sm.tile([P, 1], F32, tag="kf")
                        nc.vector.tensor_scalar(out=kf[:], in0=nn[:], scalar1=HARD_RATIO,
                                                scalar2=None, op0=ALU.mult)
                        ki = p_sm.tile([P, 1], I32, tag="ki")
                        nc.vector.tensor_copy(ki[:], kf[:])
                        kf2 = p_sm.tile([P, 1], F32, tag="kf2")
                        nc.vector.tensor_copy(kf2[:], ki[:])
                        nc.vector.tensor_scalar_max(kf2[:], kf2[:], 1.0)
                        # lognormal moment fit -> mu, sig (in s/TAU domain)
                        rnn = p_sm.tile([P, 1], F32, tag="rnn")
                        nc.vector.reciprocal(rnn[:], nn[:])
                        m1 = p_sm.tile([P, 1], F32, tag="m1")
                        nc.vector.tensor_tensor(m1[:], st["neg"][:], rnn[:], ALU.mult)
                        lnm1 = p_sm.tile([P, 1], F32, tag="lnm1")
                        nc.scalar.activation(lnm1[:], m1[:], AF.Ln)
                        m2m = p_sm.tile([P, 1], F32, tag="m2m")
                        nc.vector.tensor_tensor(m2m[:], st["m2"][:], rnn[:], ALU.mult)
                        lnm2 = p_sm.tile([P, 1], F32, tag="lnm2")
                        nc.scalar.activation(lnm2[:], m2m[:], AF.Ln)
                        sig2 = p_sm.tile([P, 1], F32, tag="sig2")
                        nc.vector.tensor_scalar(out=sig2[:], in0=lnm1[:], scalar1=-2.0,
                                                scalar2=None, op0=ALU.mult)
                        nc.vector.tensor_tensor(sig2[:], sig2[:], lnm2[:], ALU.add)
                        nc.vector.tensor_scalar_max(sig2[:], sig2[:], 1e-8)
                        sig = p_sm.tile([P, 1], F32, tag="sig")
                        nc.scalar.activation(sig[:], sig2[:], AF.Sqrt)
                        mu = p_sm.tile([P, 1], F32, tag="mu")
                        nc.vector.tensor_scalar(out=mu[:], in0=lnm2[:], scalar1=-0.5,
                                                scalar2=None, op0=ALU.mult)
                        t2 = p_sm.tile([P, 1], F32, tag="t2")
                        nc.vector.tensor_scalar(out=t2[:], in0=lnm1[:], scalar1=2.0,
                                                scalar2=None, op0=ALU.mult)
                        nc.vector.tensor_tensor(mu[:], mu[:], t2[:], ALU.add)
                        # z = Z0 + (k/nneg - HARD_RATIO) * DZDP ; t0 = mu + sig*z
                        pr = p_sm.tile([P, 1], F32, tag="pr")
                        nc.vector.tensor_tensor(pr[:], kf2[:], rnn[:], ALU.mult)
                        z = p_sm.tile([P, 1], F32, tag="z")
                        nc.vector.tensor_scalar(out=z[:], in0=pr[:], scalar1=-HARD_RATIO,
                                                scalar2=DZDP, op0=ALU.add, op1=ALU.mult)
                        nc.vector.tensor_scalar(out=z[:], in0=z[:], scalar1=Z0,
                                                scalar2=None, op0=ALU.add)
                        t_ = p_sm.tile([P, 1], F32, tag="tthr")
                        nc.vector.tensor_tensor(t_[:], sig[:], z[:], ALU.mult)
                        nc.vector.tensor_tensor(t_[:], t_[:], mu[:], ALU.add)
                        # step = sig / (nneg * PHI0)  (model-density Newton step)
                        stp = p_sm.tile([P, 1], F32, tag="stp")
                        nc.vector.tensor_scalar(out=stp[:], in0=nn[:], scalar1=PHI0,
                                                scalar2=None, op0=ALU.mult)
                        nc.vector.reciprocal(stp[:], stp[:])
                        nc.vector.tensor_tensor(stp[:], stp[:], sig[:], ALU.mult)
                        for it in range(n_newton):
                            u = p_sm.tile([P, 1], F32, tag="u")
                            nc.scalar.activation(u[:], t_[:], AF.Exp)
                            g8 = p_cnt8.tile([P, N], U8, tag="g8")
                            cnt = p_sm.tile([P, 1], F32, tag="cnt")
                            nc.vector.tensor_scalar(out=g8[:], in0=vneg[:], scalar1=u[:],
                                                    scalar2=None, op0=ALU.is_gt,
                                                    op1=ALU.add, accum_out=cnt[:])
                            d_ = p_sm.tile([P, 1], F32, tag="dstep")
                            nc.vector.tensor_scalar(out=d_[:], in0=cnt[:], scalar1=kf2[:],
                                                    scalar2=None, op0=ALU.subtract)
                            nc.vector.tensor_tensor(d_[:], d_[:], stp[:], ALU.mult)
                            nc.vector.tensor_tensor(t_[:], t_[:], d_[:], ALU.add)
                        uf = p_sm.tile([P, 1], F32, tag="uf")
                        nc.scalar.activation(uf[:], t_[:], AF.Exp)
                        hard = p_per.tile([P, 1], F32, tag=f"st_hard_{mi}_{b}")
                        gse = p_cnt8.tile([P, N], U8, tag="g8")
                        nc.vector.scalar_tensor_tensor(
                            out=gse[:], in0=vneg[:], scalar=uf[:], in1=vneg[:],
                            op0=ALU.is_gt, op1=ALU.mult, accum_out=hard[:])
                        st["hard"] = hard
                        stats[(mi, b)] = st

                # ---------- Phase E: per-row loss ----------
                for b in range(NB):
                    sM, sP = stats[(0, b)], stats[(1, b)]
                    w0 = wblk[b][:, 0:1]
                    w1_ = wblk[b][:, 1:2]
                    posM = p_sm.tile([P, 1], F32, tag="posM")
                    nc.vector.tensor_tensor(posM[:], sM["tot"][:], sM["neg"][:], ALU.subtract)
                    posP = p_sm.tile([P, 1], F32, tag="posP")
                    nc.vector.tensor_tensor(posP[:], sP["tot"][:], sP["neg"][:], ALU.subtract)
                    allM = p_sm.tile([P, 1], F32, tag="allM")
                    nc.vector.tensor_tensor(allM[:], sM["tot"][:], sM["hard"][:], ALU.add)
                    allP = p_sm.tile([P, 1], F32, tag="allP")
                    nc.vector.tensor_tensor(allP[:], sP["tot"][:], sP["hard"][:], ALU.add)
                    wpos = p_sm.tile([P, 1], F32, tag="wpos")
                    nc.vector.tensor_tensor(wpos[:], posM[:], w0, ALU.mult)
                    tq = p_sm.tile([P, 1], F32, tag="tq")
                    nc.vector.tensor_tensor(tq[:], posP[:], w1_, ALU.mult)
                    nc.vector.tensor_tensor(wpos[:], wpos[:], tq[:], ALU.add)
                    den = p_sm.tile([P, 1], F32, tag="den")
                    nc.vector.tensor_tensor(den[:], allM[:], w0, ALU.mult)
                    tq2 = p_sm.tile([P, 1], F32, tag="tq2")
                    nc.vector.tensor_tensor(tq2[:], allP[:], w1_, ALU.mult)
                    nc.vector.tensor_tensor(den[:], den[:], tq2[:], ALU.add)
                    nc.vector.tensor_scalar_max(den[:], den[:], 1e-10)
                    rden = p_sm.tile([P, 1], F32, tag="rden")
                    nc.vector.reciprocal(rden[:], den[:])
                    ratio = p_sm.tile([P, 1], F32, tag="ratio")
                    nc.vector.tensor_tensor(ratio[:], wpos[:], rden[:], ALU.mult)
                    nei = p_sm.tile([P, 1], F32, tag="nei")
                    nc.vector.tensor_tensor(nei[:], sM["nneg"][:], sP["nneg"][:], ALU.add)
                    nc.vector.tensor_scalar(out=nei[:], in0=nei[:], scalar1=-1.0,
                                            scalar2=float(2 * N), op0=ALU.mult, op1=ALU.add)
                    nc.vector.tensor_scalar_max(nei[:], nei[:], 1.0)
                    rnei = p_sm.tile([P, 1], F32, tag="rnei")
                    nc.vector.reciprocal(rnei[:], nei[:])
                    nc.vector.tensor_tensor(ratio[:], ratio[:], rnei[:], ALU.mult)
                    nc.vector.tensor_scalar_max(ratio[:], ratio[:], 1e-10)
                    ll = p_sm.tile([P, 1], F32, tag="ll")
                    nc.scalar.activation(ll[:], ratio[:], AF.Ln)
                    nc.vector.tensor_scalar(out=ll[:], in0=ll[:], scalar1=-1.0,
                                            scalar2=None, op0=ALU.mult)
                    nc.sync.dma_start(loss_out[b * P:(b + 1) * P, :], ll[:])
    nc.finalize()
    return nc


_NC_CACHE = {}


def _get_nc(R, N, D):
    key = (R, N, D)
    if key not in _NC_CACHE:
        _NC_CACHE[key] = build(R, N, D)
    return _NC_CACHE[key]


def make_in_maps(embF, embM, embP, FM_adj, FP_adj, W1, b1, W2, b2, n_cores=8):
    import ml_dtypes
    N, D = embM.shape
    R = embF.shape[0] // n_cores
    f32 = np.float32
    bf16 = ml_dtypes.bfloat16
    embMT = np.ascontiguousarray(embM.T, dtype=f32)
    embPT = np.ascontiguousarray(embP.T, dtype=f32)
    adjTM = np.ascontiguousarray(FM_adj.T).astype(bf16)
    adjTP = np.ascontiguousarray(FP_adj.T).astype(bf16)
    negadjM = (1.0 - FM_adj).astype(bf16)
    negadjP = (1.0 - FP_adj).astype(bf16)
    ident = np.eye(128, dtype=f32)
    embM = np.asarray(embM, f32)
    embP = np.asarray(embP, f32)
    embM_hi = embM.astype(bf16)
    embM_lo = (embM - embM_hi.astype(f32)).astype(bf16)
    embP_hi = embP.astype(bf16)
    embP_lo = (embP - embP_hi.astype(f32)).astype(bf16)
    W1 = np.ascontiguousarray(W1, dtype=f32)
    b1 = np.ascontiguousarray(np.asarray(b1).reshape(-1, 1), dtype=f32)
    W2 = np.ascontiguousarray(W2, dtype=f32)
    b2 = np.ascontiguousarray(np.asarray(b2).reshape(-1, 1), dtype=f32)
    in_maps = []
    for c in range(n_cores):
        sl = slice(c * R, (c + 1) * R)
        in_maps.append(dict(
            embF=np.ascontiguousarray(embF[sl], dtype=f32),
            embMT=embMT, embPT=embPT,
            embM_hi=embM_hi, embM_lo=embM_lo, embP_hi=embP_hi, embP_lo=embP_lo,
            adjTM=np.ascontiguousarray(adjTM[:, sl]),
            adjTP=np.ascontiguousarray(adjTP[:, sl]),
            negadjM=np.ascontiguousarray(negadjM[sl]),
            negadjP=np.ascontiguousarray(negadjP[sl]),
            W1=W1, b1=b1, W2=W2, b2=b2, ident=ident,
        ))
    return in_maps


def kernel(embF, embM, embP, FM_adj, FP_adj, W1, b1, W2, b2):
    from concourse.bass_utils import run_bass_kernel_spmd
    n_cores = 8
    embF = np.asarray(embF, np.float32)
    N, D = np.asarray(embM).shape
    R = embF.shape[0] // n_cores
    nc = _get_nc(R, N, D)
    in_maps = make_in_maps(embF, np.asarray(embM), np.asarray(embP),
                           np.asarray(FM_adj, np.float32),
                           np.asarray(FP_adj, np.float32),
                           np.asarray(W1), np.asarray(b1), np.asarray(W2),
                           np.asarray(b2), n_cores)
    res = run_bass_kernel_spmd(nc, in_maps, list(range(n_cores)))
    loss_rows = np.concatenate([res.results[c]["loss_rows"][:, 0] for c in range(n_cores)])
    wts = np.concatenate([res.results[c]["wtsT"].T for c in range(n_cores)], axis=0)
    loss = np.float32(np.mean(loss_rows, dtype=np.float64))
    return np.asarray(loss, np.float32), wts.astype(np.float32)
